# revision 1
# baseline (speedup 1.0000x reference)
import numpy as np
import jax
import jax.numpy as jnp
from functools import partial

# DCN v2 forward, reformulated gather-free:
# bilinear sampling at p+u (|u|<=2) == sum_{d in -2..2} tent(u-d) * x[p+d]
# (exact piecewise-linear interpolation; tent(t) = relu(1-|t|)).
# Out-of-image samples contribute zero, which zero-padding implements exactly.

B, C, O, H, W = 8, 64, 64, 128, 128
K = 9
D = 5  # window taps per axis: d in {-2,-1,0,1,2}


def _conv3x3(x, w, b):
    y = jax.lax.conv_general_dilated(
        x, w, window_strides=(1, 1), padding=((1, 1), (1, 1)),
        dimension_numbers=('NCHW', 'OIHW', 'NCHW'))
    return y + b[None, :, None, None]


def _dcn_shard(x, w_offset, b_offset, w_mod, b_mod, w_conv, b_conv):
    # x: [b, C, H, W] for this shard
    b = x.shape[0]
    off = _conv3x3(x, w_offset, b_offset).reshape(b, K, 2, H, W)
    mask = 2.0 * jax.nn.sigmoid(_conv3x3(x, w_mod, b_mod))  # [b,K,H,W]
    oy = off[:, :, 0]  # [b,K,H,W]
    ox = off[:, :, 1]

    # tent weights per axis: wy[d] = relu(1 - |oy - d|), d in {-2..2}
    ds = jnp.arange(-2, 3, dtype=x.dtype)
    wy = jax.nn.relu(1.0 - jnp.abs(oy[:, :, None] - ds[None, None, :, None, None]))
    wx = jax.nn.relu(1.0 - jnp.abs(ox[:, :, None] - ds[None, None, :, None, None]))
    wy = wy * mask[:, :, None]  # fold modulation into y-weights [b,K,D,H,W]

    # padded input: sample rows/cols i-1+ky+dy for dy in -2..2 -> i + (ky+dy-1),
    # range of total shift per axis: ky-1+dy in [-3, 3] -> pad 3+3
    P = 3
    xp = jnp.pad(x, ((0, 0), (0, 0), (P, P), (P, P)))

    k = np.arange(K)
    ky = k // 3
    kx = k % 3

    out = jnp.zeros((b, O, H, W), x.dtype)
    wf = w_conv.reshape(O, C, K)
    for ki in range(K):
        # accumulate sampled*mask for tap ki: s[b,c,h,w]
        s = jnp.zeros((b, C, H, W), x.dtype)
        for dy in range(D):
            ry = P + int(ky[ki]) - 1 + dy - 2  # row offset into xp
            xrow = jax.lax.dynamic_slice_in_dim(xp, ry, H, axis=2)
            srow = jnp.zeros((b, C, H, W), x.dtype)
            for dx in range(D):
                rx = P + int(kx[ki]) - 1 + dx - 2
                xwin = jax.lax.dynamic_slice_in_dim(xrow, rx, W, axis=3)
                srow = srow + wx[:, ki, dx, None] * xwin
            s = s + wy[:, ki, dy, None] * srow
        out = out + jnp.einsum('oc,bchw->bohw', wf[:, :, ki], s)
    return out + b_conv[None, :, None, None]


def kernel(x, w_offset, b_offset, w_mod, b_mod, w_conv, b_conv):
    x = np.asarray(x, dtype=np.float32)
    devs = jax.devices()[:8]
    fn = jax.pmap(_dcn_shard, axis_name='i', in_axes=(0, None, None, None, None, None, None),
                  devices=devs)
    xs = x.reshape(8, B // 8, C, H, W)
    out = fn(xs, jnp.asarray(w_offset), jnp.asarray(b_offset), jnp.asarray(w_mod),
             jnp.asarray(b_mod), jnp.asarray(w_conv), jnp.asarray(b_conv))
    return np.asarray(out).reshape(B, O, H, W)

